# Initial kernel scaffold
#
"""CoxPHLoss v2b: fp8 events-first layout + TINY collectives.

AllGather of per-core scalar total S1 (fires right after the exp pass,
overlapping all per-bin vector reduces), local-bins-only epilogue, then a
tiny AllReduce of [partial_mse, E]. S2 via native STT (the custom
tensor_tensor_reduce op crashes the HW worker).
"""

import os
import numpy as np

N = 8_000_000
K = 10_000
NCORES = 8
BINS_PER_SHARD = K // NCORES
R = 1280
NCHUNK = R // 128
PAD = -240.0
C1_DEFAULT = 480
C0_DEFAULT = 480

LAST_EXEC_TIME_NS = None
LAST_RESULTS = None
TRACE = bool(int(os.environ.get("KERNEL_TRACE", "0")))

_CACHE = {}


def _build_program(C1: int, C0: int):
    import concourse.bacc as bacc
    import concourse.mybir as mybir
    import concourse.tile as tile

    f32 = mybir.dt.float32
    bf16 = mybir.dt.bfloat16
    fp8 = mybir.dt.float8e4
    Alu = mybir.AluOpType
    Act = mybir.ActivationFunctionType
    Ax = mybir.AxisListType
    C = C1 + C0

    nc = bacc.Bacc("TRN2", target_bir_lowering=False, debug=False,
                   num_devices=NCORES)

    x_d = nc.dram_tensor("x_d", [R, C], fp8, kind="ExternalInput")
    cnt_d = nc.dram_tensor("cnt_d", [R, 1], f32, kind="ExternalInput")
    mask_d = nc.dram_tensor("mask_d", [8, 1], f32, kind="ExternalInput")
    mse_d = nc.dram_tensor("mse_d", [1, 1], f32, kind="ExternalOutput")

    x_v = x_d.ap().rearrange("(a p) w -> p a w", p=128)
    cnt_v = cnt_d.ap().rearrange("(a p) w -> p (a w)", p=128)

    tril_inc_h = nc.inline_tensor(
        np.tril(np.ones((128, 128), np.float32)), name="tril_inc")
    tril_str_h = nc.inline_tensor(
        np.tril(np.ones((128, 128), np.float32), -1), name="tril_str")
    allones_h = nc.inline_tensor(np.ones((128, 128), np.float32), name="allones")
    ones_h = nc.inline_tensor(np.ones((128, 1), np.float32), name="ones128")

    PIECES = [(0, 1), (1, 4), (4, 7), (7, 10)]

    with tile.TileContext(nc) as tc:
        with (
            tc.tile_pool(name="io", bufs=1) as io_pool,
            tc.tile_pool(name="scr", bufs=2) as scr_pool,
            tc.tile_pool(name="small", bufs=1) as small_pool,
            tc.tile_pool(name="psum", bufs=1, space="PSUM") as psum_pool,
            tc.tile_pool(name="dram", bufs=1, space="DRAM") as dram_pool,
        ):
            x_all = io_pool.tile([128, NCHUNK, C], fp8, tag="x")
            g_all = io_pool.tile([128, NCHUNK, C], bf16, tag="g")

            for (a0, a1) in PIECES:
                nc.sync.dma_start(x_all[:, a0:a1, :], x_v[:, a0:a1, :])

            # constants + tiny inputs (issued after x; needed only late)
            tril_inc_t = small_pool.tile([128, 128], f32, tag="c0")
            tril_str_t = small_pool.tile([128, 128], f32, tag="c1")
            allones_t = small_pool.tile([128, 128], f32, tag="c2")
            ones_t = small_pool.tile([128, 1], f32, tag="c3")
            nc.sync.dma_start(tril_inc_t[:], tril_inc_h.ap())
            nc.sync.dma_start(tril_str_t[:], tril_str_h.ap())
            nc.sync.dma_start(allones_t[:], allones_h.ap())
            nc.sync.dma_start(ones_t[:], ones_h.ap())
            cntv = small_pool.tile([128, NCHUNK], f32, tag="cnt")
            nc.sync.dma_start(cntv[:], cnt_v)
            mk = small_pool.tile([128, 1], f32, tag="mk")
            nc.vector.memset(mk[:], 0.0)
            nc.sync.dma_start(mk[32:40, 0:1], mask_d.ap())

            # exp with per-piece scalar-engine accumulators
            accP = small_pool.tile([128, len(PIECES)], f32, tag="accP")
            for i, (a0, a1) in enumerate(PIECES):
                nc.scalar.activation(
                    out=g_all[:, a0:a1, :].rearrange("p a w -> p (a w)"),
                    in_=x_all[:, a0:a1, :].rearrange("p a w -> p (a w)"),
                    func=Act.Exp, accum_out=accP[:, i:i + 1])

            # per-core total S1 -> AllGather (overlaps vector stat work)
            accsum = small_pool.tile([128, 1], f32, tag="accsum")
            nc.vector.tensor_reduce(out=accsum[:], in_=accP[:], axis=Ax.X,
                                    op=Alu.add)
            tot_ps = psum_pool.tile([1, 1], f32, space="PSUM", tag="tot")
            nc.tensor.matmul(out=tot_ps[:], lhsT=ones_t[:], rhs=accsum[:],
                             start=True, stop=True)
            tots = small_pool.tile([1, 128], f32, tag="tots")
            nc.vector.memset(tots[:], 0.0)
            nc.vector.tensor_copy(out=tots[:, 0:1], in_=tot_ps[:])
            cc_in = dram_pool.tile([1, 128], f32)
            cc_out = dram_pool.tile([8, 128], f32, addr_space="Shared")
            nc.sync.dma_start(cc_in[:], tots[:])
            nc.gpsimd.collective_compute(
                "AllGather", Alu.bypass,
                replica_groups=[list(range(NCORES))],
                ins=[cc_in.opt()], outs=[cc_out.opt()])

            # per-bin stats while the collective runs
            Tt = small_pool.tile([128, NCHUNK], f32, tag="T")
            Ut = small_pool.tile([128, NCHUNK], f32, tag="U")
            S2 = small_pool.tile([128, NCHUNK], f32, tag="S2")
            for (a0, a1) in PIECES:
                nc.vector.tensor_reduce(
                    out=Tt[:, a0:a1], in_=g_all[:, a0:a1, 0:C1], axis=Ax.X,
                    op=Alu.add)
                nc.vector.tensor_reduce(
                    out=Ut[:, a0:a1], in_=g_all[:, a0:a1, C1:C], axis=Ax.X,
                    op=Alu.add)
            for a in range(NCHUNK):
                junk = scr_pool.tile([128, C], bf16, tag="junk")
                nc.vector.scalar_tensor_tensor(
                    out=junk[:], in0=g_all[:, a, :], scalar=1.0,
                    in1=g_all[:, a, :], op0=Alu.mult, op1=Alu.mult,
                    accum_out=S2[:, a:a + 1])
            S1 = small_pool.tile([128, NCHUNK], f32, tag="S1")
            nc.vector.tensor_tensor(out=S1[:], in0=Tt[:], in1=Ut[:],
                                    op=Alu.add)

            # local suffix-cumsum over own 1280 bins (bin = 128*a + p)
            cw_ps = psum_pool.tile([128, NCHUNK], f32, space="PSUM", tag="cw")
            nc.tensor.matmul(out=cw_ps[:], lhsT=tril_inc_t[:], rhs=S1[:],
                             start=True, stop=True)
            cws = small_pool.tile([128, NCHUNK], f32, tag="cws")
            nc.vector.tensor_copy(out=cws[:], in_=cw_ps[:])
            totT_ps = psum_pool.tile([NCHUNK, 1], f32, space="PSUM", tag="tt")
            nc.tensor.matmul(out=totT_ps[:], lhsT=S1[:], rhs=ones_t[:],
                             start=True, stop=True)
            totT = small_pool.tile([128, 1], f32, tag="totT")
            nc.vector.memset(totT[:], 0.0)
            nc.vector.tensor_copy(out=totT[0:NCHUNK, :], in_=totT_ps[:])
            rr = small_pool.tile([128, NCHUNK], f32, tag="rr")
            nc.vector.tensor_tensor(
                out=rr[:], in0=tril_str_t[:, 0:NCHUNK],
                in1=totT[:, 0:1].to_broadcast([128, NCHUNK]), op=Alu.mult)

            w = small_pool.tile([128, 2], f32, tag="w")
            nc.vector.tensor_reduce(out=w[:, 1:2], in_=cntv[:], axis=Ax.X,
                                    op=Alu.add)

            # gathered totals * suffix mask -> rows 32:40 of rr
            tg = small_pool.tile([128, 1], f32, tag="tg")
            nc.vector.memset(tg[:], 0.0)
            nc.sync.dma_start(tg[32:40, 0:1], cc_out.opt()[:, 0:1])
            nc.vector.tensor_tensor(
                out=rr[32:40, :], in0=tg[32:40, 0:1].to_broadcast([8, NCHUNK]),
                in1=mk[32:40, 0:1].to_broadcast([8, NCHUNK]), op=Alu.mult)

            roff_ps = psum_pool.tile([128, NCHUNK], f32, space="PSUM", tag="ro")
            nc.tensor.matmul(out=roff_ps[:], lhsT=allones_t[:], rhs=rr[:],
                             start=True, stop=True)
            risk = small_pool.tile([128, NCHUNK], f32, tag="risk")
            nc.vector.tensor_tensor(out=risk[:], in0=cws[:], in1=roff_ps[:],
                                    op=Alu.add)
            nc.vector.tensor_scalar_max(risk[:], risk[:], 1e-30)
            rrec = small_pool.tile([128, NCHUNK], f32, tag="rrec")
            nc.vector.reciprocal(rrec[:], risk[:])
            base = small_pool.tile([128, NCHUNK], f32, tag="base")
            nc.vector.tensor_tensor(out=base[:], in0=cntv[:], in1=rrec[:],
                                    op=Alu.mult)

            # partial = sum_k [cntE + base^2*S2 - 2*base*T] over own bins
            t1 = small_pool.tile([128, NCHUNK], f32, tag="t1")
            nc.vector.tensor_tensor(out=t1[:], in0=base[:], in1=S2[:],
                                    op=Alu.mult)
            t2 = small_pool.tile([128, NCHUNK], f32, tag="t2")
            nc.vector.scalar_tensor_tensor(
                out=t2[:], in0=Tt[:], scalar=-2.0, in1=t1[:],
                op0=Alu.mult, op1=Alu.add)
            junk2 = small_pool.tile([128, NCHUNK], f32, tag="junk2")
            nc.vector.scalar_tensor_tensor(
                out=junk2[:], in0=base[:], scalar=1.0, in1=t2[:],
                op0=Alu.mult, op1=Alu.mult, accum_out=w[:, 0:1])

            wsum = small_pool.tile([128, 2], f32, tag="wsum")
            nc.vector.tensor_tensor(out=wsum[:, 0:1], in0=w[:, 0:1],
                                    in1=w[:, 1:2], op=Alu.add)
            nc.vector.tensor_copy(out=wsum[:, 1:2], in_=w[:, 1:2])
            fin_ps = psum_pool.tile([1, 2], f32, space="PSUM", tag="fin")
            nc.tensor.matmul(out=fin_ps[:], lhsT=ones_t[:], rhs=wsum[:],
                             start=True, stop=True)
            finv = small_pool.tile([1, 128], f32, tag="finv")
            nc.vector.memset(finv[:], 0.0)
            nc.vector.tensor_copy(out=finv[:, 0:2], in_=fin_ps[:])
            ar_in = dram_pool.tile([1, 128], f32)
            ar_out = dram_pool.tile([8, 128], f32, addr_space="Shared")
            nc.sync.dma_start(ar_in[:], finv[:])
            nc.gpsimd.collective_compute(
                "AllGather", Alu.bypass,
                replica_groups=[list(range(NCORES))],
                ins=[ar_in.opt()], outs=[ar_out.opt()])
            # sum the 8 gathered [partial, E] rows with an 8-partition matmul
            pg = small_pool.tile([128, 2], f32, tag="pg")
            nc.sync.dma_start(pg[0:8, 0:2], ar_out.opt()[:, 0:2])
            fin2_ps = psum_pool.tile([1, 2], f32, space="PSUM", tag="fin2")
            nc.tensor.matmul(out=fin2_ps[:], lhsT=ones_t[0:8, :],
                             rhs=pg[0:8, :], start=True, stop=True)

            q = small_pool.tile([1, 1], f32, tag="q")
            nc.vector.tensor_scalar_mul(q[:], fin2_ps[0:1, 0:1], 1.0 / N)
            gate = small_pool.tile([1, 1], f32, tag="gate")
            nc.vector.tensor_scalar_min(gate[:], fin2_ps[0:1, 1:2], 1.0)
            mse_t = small_pool.tile([1, 1], f32, tag="mse")
            nc.vector.tensor_tensor(out=mse_t[:], in0=q[:], in1=gate[:],
                                    op=Alu.mult)
            nc.sync.dma_start(mse_d.ap(), mse_t[:])

    nc.compile()
    return nc


def _shard_inputs(log_h, durations, events, C1, C0):
    import ml_dtypes

    C = C1 + C0
    d = durations.astype(np.int64, copy=False)
    e = events.astype(np.int64, copy=False)
    order = np.argsort(d * 2 + (1 - e), kind="stable")
    d_s = d[order]
    cnt_all = np.bincount(d, minlength=K)
    cntE = np.bincount(d[e == 1], minlength=K)
    starts = np.zeros(K, np.int64)
    starts[1:] = np.cumsum(cnt_all)[:-1]
    pos = np.arange(N, dtype=np.int64) - starts[d_s]
    is_ev = pos < cntE[d_s]
    col = np.where(is_ev, pos, C1 + (pos - cntE[d_s]))
    rows = (d_s // BINS_PER_SHARD) * R + (d_s % BINS_PER_SHARD)

    f8 = ml_dtypes.float8_e4m3fn
    X = np.full((NCORES * R, C), PAD, dtype=f8)
    X[rows, col] = log_h[order].astype(f8)

    cnt_rows = np.zeros(NCORES * R, np.float32)
    bins = np.arange(K, dtype=np.int64)
    cnt_rows[(bins // BINS_PER_SHARD) * R + (bins % BINS_PER_SHARD)] = cntE

    in_maps = []
    for s in range(NCORES):
        msk = np.zeros((8, 1), np.float32)
        msk[s + 1:, 0] = 1.0
        in_maps.append({
            "x_d": np.ascontiguousarray(X[s * R:(s + 1) * R]),
            "cnt_d": np.ascontiguousarray(
                cnt_rows[s * R:(s + 1) * R].reshape(R, 1)),
            "mask_d": msk,
        })
    return in_maps


def kernel(log_h, durations, events):
    global LAST_EXEC_TIME_NS, LAST_RESULTS
    from concourse.bass_utils import run_bass_kernel_spmd

    assert log_h.shape == (N,) and durations.shape == (N,)

    d64 = durations.astype(np.int64, copy=False)
    e64 = events.astype(np.int64, copy=False)
    cntE = np.bincount(d64[e64 == 1], minlength=K)
    cntO = np.bincount(d64[e64 == 0], minlength=K)
    C1 = max(C1_DEFAULT, int(-(-cntE.max() // 16) * 16))
    C0 = max(C0_DEFAULT, int(-(-cntO.max() // 16) * 16))

    if (C1, C0) not in _CACHE:
        _CACHE[(C1, C0)] = _build_program(C1, C0)
    nc = _CACHE[(C1, C0)]

    in_maps = _shard_inputs(log_h, durations, events, C1, C0)
    res = run_bass_kernel_spmd(
        nc, in_maps, core_ids=list(range(NCORES)), trace=TRACE,
        trace_cores=None)
    LAST_EXEC_TIME_NS = res.exec_time_ns
    LAST_RESULTS = res
    mse = res.results[0]["mse_d"][0, 0]
    return np.asarray(mse, dtype=np.float32).reshape(())



# revision 1
# speedup vs baseline: 1.0797x; 1.0797x over previous
"""CoxPHLoss v2b: fp8 events-first layout + TINY collectives.

AllGather of per-core scalar total S1 (fires right after the exp pass,
overlapping all per-bin vector reduces), local-bins-only epilogue, then a
tiny AllReduce of [partial_mse, E]. S2 via native STT (the custom
tensor_tensor_reduce op crashes the HW worker).
"""

import os
import numpy as np

N = 8_000_000
K = 10_000
NCORES = 8
BINS_PER_SHARD = K // NCORES
R = 1280
NCHUNK = R // 128
PAD = -240.0
C1_DEFAULT = 480
C0_DEFAULT = 480

LAST_EXEC_TIME_NS = None
LAST_RESULTS = None
TRACE = bool(int(os.environ.get("KERNEL_TRACE", "0")))

_CACHE = {}


def _build_program(C1: int, C0: int):
    import concourse.bacc as bacc
    import concourse.mybir as mybir
    import concourse.tile as tile

    f32 = mybir.dt.float32
    bf16 = mybir.dt.bfloat16
    fp8 = mybir.dt.float8e4
    Alu = mybir.AluOpType
    Act = mybir.ActivationFunctionType
    Ax = mybir.AxisListType
    C = C1 + C0

    nc = bacc.Bacc("TRN2", target_bir_lowering=False, debug=False,
                   num_devices=NCORES)

    x_d = nc.dram_tensor("x_d", [R, C], fp8, kind="ExternalInput")
    cnt_d = nc.dram_tensor("cnt_d", [R, 1], f32, kind="ExternalInput")
    mask_d = nc.dram_tensor("mask_d", [8, 1], f32, kind="ExternalInput")
    mse_d = nc.dram_tensor("mse_d", [1, 1], f32, kind="ExternalOutput")

    x_v = x_d.ap().rearrange("(a p) w -> p a w", p=128)
    cnt_v = cnt_d.ap().rearrange("(a p) w -> p (a w)", p=128)

    tril_inc_h = nc.inline_tensor(
        np.tril(np.ones((128, 128), np.float32)), name="tril_inc")
    tril_str_h = nc.inline_tensor(
        np.tril(np.ones((128, 128), np.float32), -1), name="tril_str")
    allones_h = nc.inline_tensor(np.ones((128, 128), np.float32), name="allones")
    ones_h = nc.inline_tensor(np.ones((128, 1), np.float32), name="ones128")

    PIECES = [(0, 1), (1, 4), (4, 7), (7, 10)]

    with tile.TileContext(nc) as tc:
        with (
            tc.tile_pool(name="io", bufs=1) as io_pool,
            tc.tile_pool(name="scr", bufs=2) as scr_pool,
            tc.tile_pool(name="small", bufs=1) as small_pool,
            tc.tile_pool(name="psum", bufs=1, space="PSUM") as psum_pool,
            tc.tile_pool(name="dram", bufs=1, space="DRAM") as dram_pool,
        ):
            x_all = io_pool.tile([128, NCHUNK, C], fp8, tag="x")
            g_all = io_pool.tile([128, NCHUNK, C], bf16, tag="g")

            for (a0, a1) in PIECES:
                nc.sync.dma_start(x_all[:, a0:a1, :], x_v[:, a0:a1, :])

            # constants + tiny inputs (issued after x; needed only late)
            tril_inc_t = small_pool.tile([128, 128], f32, tag="c0")
            tril_str_t = small_pool.tile([128, 128], f32, tag="c1")
            allones_t = small_pool.tile([128, 128], f32, tag="c2")
            ones_t = small_pool.tile([128, 1], f32, tag="c3")
            nc.sync.dma_start(tril_inc_t[:], tril_inc_h.ap())
            nc.sync.dma_start(tril_str_t[:], tril_str_h.ap())
            nc.sync.dma_start(allones_t[:], allones_h.ap())
            nc.sync.dma_start(ones_t[:], ones_h.ap())
            cntv = small_pool.tile([128, NCHUNK], f32, tag="cnt")
            nc.sync.dma_start(cntv[:], cnt_v)
            mk = small_pool.tile([128, 1], f32, tag="mk")
            nc.vector.memset(mk[:], 0.0)
            nc.sync.dma_start(mk[32:40, 0:1], mask_d.ap())

            # exp with per-piece scalar-engine accumulators
            accP = small_pool.tile([128, len(PIECES)], f32, tag="accP")
            for i, (a0, a1) in enumerate(PIECES):
                nc.scalar.activation(
                    out=g_all[:, a0:a1, :].rearrange("p a w -> p (a w)"),
                    in_=x_all[:, a0:a1, :].rearrange("p a w -> p (a w)"),
                    func=Act.Exp, accum_out=accP[:, i:i + 1])

            # per-core total S1 -> AllGather (overlaps vector stat work)
            accsum = small_pool.tile([128, 1], f32, tag="accsum")
            nc.vector.tensor_reduce(out=accsum[:], in_=accP[:], axis=Ax.X,
                                    op=Alu.add)
            tot_ps = psum_pool.tile([1, 1], f32, space="PSUM", tag="tot")
            nc.tensor.matmul(out=tot_ps[:], lhsT=ones_t[:], rhs=accsum[:],
                             start=True, stop=True)
            tots = small_pool.tile([1, 128], f32, tag="tots")
            nc.vector.memset(tots[:], 0.0)
            nc.vector.tensor_copy(out=tots[:, 0:1], in_=tot_ps[:])
            cc_in = dram_pool.tile([1, 128], f32)
            cc_out = dram_pool.tile([8, 128], f32, addr_space="Shared")
            nc.sync.dma_start(cc_in[:], tots[:])
            nc.gpsimd.collective_compute(
                "AllGather", Alu.bypass,
                replica_groups=[list(range(NCORES))],
                ins=[cc_in.opt()], outs=[cc_out.opt()])

            # per-bin stats while the collective runs
            Tt = small_pool.tile([128, NCHUNK], f32, tag="T")
            Ut = small_pool.tile([128, NCHUNK], f32, tag="U")
            S2 = small_pool.tile([128, NCHUNK], f32, tag="S2")
            for (a0, a1) in PIECES:
                nc.vector.tensor_reduce(
                    out=Tt[:, a0:a1], in_=g_all[:, a0:a1, 0:C1], axis=Ax.X,
                    op=Alu.add)
                nc.vector.tensor_reduce(
                    out=Ut[:, a0:a1], in_=g_all[:, a0:a1, C1:C], axis=Ax.X,
                    op=Alu.add)
            for a in range(NCHUNK):
                junk = scr_pool.tile([128, C], bf16, tag="junk")
                nc.vector.scalar_tensor_tensor(
                    out=junk[:], in0=g_all[:, a, :], scalar=1.0,
                    in1=g_all[:, a, :], op0=Alu.mult, op1=Alu.mult,
                    accum_out=S2[:, a:a + 1])
            S1 = small_pool.tile([128, NCHUNK], f32, tag="S1")
            nc.vector.tensor_tensor(out=S1[:], in0=Tt[:], in1=Ut[:],
                                    op=Alu.add)

            # local suffix-cumsum over own 1280 bins (bin = 128*a + p)
            cw_ps = psum_pool.tile([128, NCHUNK], f32, space="PSUM", tag="cw")
            nc.tensor.matmul(out=cw_ps[:], lhsT=tril_inc_t[:], rhs=S1[:],
                             start=True, stop=True)
            cws = small_pool.tile([128, NCHUNK], f32, tag="cws")
            nc.vector.tensor_copy(out=cws[:], in_=cw_ps[:])
            totT_ps = psum_pool.tile([NCHUNK, 1], f32, space="PSUM", tag="tt")
            nc.tensor.matmul(out=totT_ps[:], lhsT=S1[:], rhs=ones_t[:],
                             start=True, stop=True)
            totT = small_pool.tile([128, 1], f32, tag="totT")
            nc.vector.memset(totT[:], 0.0)
            nc.vector.tensor_copy(out=totT[0:NCHUNK, :], in_=totT_ps[:])
            rr = small_pool.tile([128, NCHUNK], f32, tag="rr")
            nc.vector.tensor_tensor(
                out=rr[:], in0=tril_str_t[:, 0:NCHUNK],
                in1=totT[:, 0:1].to_broadcast([128, NCHUNK]), op=Alu.mult)

            w = small_pool.tile([128, 2], f32, tag="w")
            nc.vector.tensor_reduce(out=w[:, 1:2], in_=cntv[:], axis=Ax.X,
                                    op=Alu.add)

            # gathered totals * suffix mask -> rows 32:40 of rr
            tg = small_pool.tile([128, 1], f32, tag="tg")
            nc.vector.memset(tg[:], 0.0)
            nc.sync.dma_start(tg[32:40, 0:1], cc_out.opt()[:, 0:1])
            nc.vector.tensor_tensor(
                out=rr[32:40, :], in0=tg[32:40, 0:1].to_broadcast([8, NCHUNK]),
                in1=mk[32:40, 0:1].to_broadcast([8, NCHUNK]), op=Alu.mult)

            roff_ps = psum_pool.tile([128, NCHUNK], f32, space="PSUM", tag="ro")
            nc.tensor.matmul(out=roff_ps[:], lhsT=allones_t[:], rhs=rr[:],
                             start=True, stop=True)
            risk = small_pool.tile([128, NCHUNK], f32, tag="risk")
            nc.vector.tensor_tensor(out=risk[:], in0=cws[:], in1=roff_ps[:],
                                    op=Alu.add)
            nc.vector.tensor_scalar_max(risk[:], risk[:], 1e-30)
            rrec = small_pool.tile([128, NCHUNK], f32, tag="rrec")
            nc.vector.reciprocal(rrec[:], risk[:])
            base = small_pool.tile([128, NCHUNK], f32, tag="base")
            nc.vector.tensor_tensor(out=base[:], in0=cntv[:], in1=rrec[:],
                                    op=Alu.mult)

            # partial = sum_k [cntE + base^2*S2 - 2*base*T] over own bins
            t1 = small_pool.tile([128, NCHUNK], f32, tag="t1")
            nc.vector.tensor_tensor(out=t1[:], in0=base[:], in1=S2[:],
                                    op=Alu.mult)
            t2 = small_pool.tile([128, NCHUNK], f32, tag="t2")
            nc.vector.scalar_tensor_tensor(
                out=t2[:], in0=Tt[:], scalar=-2.0, in1=t1[:],
                op0=Alu.mult, op1=Alu.add)
            junk2 = small_pool.tile([128, NCHUNK], f32, tag="junk2")
            nc.vector.scalar_tensor_tensor(
                out=junk2[:], in0=base[:], scalar=1.0, in1=t2[:],
                op0=Alu.mult, op1=Alu.mult, accum_out=w[:, 0:1])

            wsum = small_pool.tile([128, 2], f32, tag="wsum")
            nc.vector.tensor_tensor(out=wsum[:, 0:1], in0=w[:, 0:1],
                                    in1=w[:, 1:2], op=Alu.add)
            nc.vector.tensor_copy(out=wsum[:, 1:2], in_=w[:, 1:2])
            fin_ps = psum_pool.tile([1, 2], f32, space="PSUM", tag="fin")
            nc.tensor.matmul(out=fin_ps[:], lhsT=ones_t[:], rhs=wsum[:],
                             start=True, stop=True)
            finv = small_pool.tile([1, 128], f32, tag="finv")
            nc.vector.memset(finv[:], 0.0)
            nc.vector.tensor_copy(out=finv[:, 0:2], in_=fin_ps[:])
            ar_in = dram_pool.tile([1, 128], f32)
            ar_out = dram_pool.tile([8, 128], f32, addr_space="Shared")
            nc.sync.dma_start(ar_in[:], finv[:])
            nc.gpsimd.collective_compute(
                "AllGather", Alu.bypass,
                replica_groups=[list(range(NCORES))],
                ins=[ar_in.opt()], outs=[ar_out.opt()])
            # sum the 8 gathered [partial, E] rows with an 8-partition matmul
            pg = small_pool.tile([128, 2], f32, tag="pg")
            nc.sync.dma_start(pg[0:8, 0:2], ar_out.opt()[:, 0:2])
            fin2_ps = psum_pool.tile([1, 2], f32, space="PSUM", tag="fin2")
            nc.tensor.matmul(out=fin2_ps[:], lhsT=ones_t[0:8, :],
                             rhs=pg[0:8, :], start=True, stop=True)

            q = small_pool.tile([1, 1], f32, tag="q")
            nc.vector.tensor_scalar_mul(q[:], fin2_ps[0:1, 0:1], 1.0 / N)
            gate = small_pool.tile([1, 1], f32, tag="gate")
            nc.vector.tensor_scalar_min(gate[:], fin2_ps[0:1, 1:2], 1.0)
            mse_t = small_pool.tile([1, 1], f32, tag="mse")
            nc.vector.tensor_tensor(out=mse_t[:], in0=q[:], in1=gate[:],
                                    op=Alu.mult)
            nc.sync.dma_start(mse_d.ap(), mse_t[:])

    nc.compile()
    return nc


def _shard_inputs(log_h, durations, events, C1, C0):
    import ml_dtypes

    C = C1 + C0
    d = durations.astype(np.int64, copy=False)
    e = events.astype(np.int64, copy=False)
    order = np.argsort(d * 2 + (1 - e), kind="stable")
    d_s = d[order]
    cnt_all = np.bincount(d, minlength=K)
    cntE = np.bincount(d[e == 1], minlength=K)
    starts = np.zeros(K, np.int64)
    starts[1:] = np.cumsum(cnt_all)[:-1]
    pos = np.arange(N, dtype=np.int64) - starts[d_s]
    is_ev = pos < cntE[d_s]
    col = np.where(is_ev, pos, C1 + (pos - cntE[d_s]))
    rows = (d_s // BINS_PER_SHARD) * R + (d_s % BINS_PER_SHARD)

    f8 = ml_dtypes.float8_e4m3fn
    X = np.full((NCORES * R, C), PAD, dtype=f8)
    X[rows, col] = log_h[order].astype(f8)

    cnt_rows = np.zeros(NCORES * R, np.float32)
    bins = np.arange(K, dtype=np.int64)
    cnt_rows[(bins // BINS_PER_SHARD) * R + (bins % BINS_PER_SHARD)] = cntE

    in_maps = []
    for s in range(NCORES):
        msk = np.zeros((8, 1), np.float32)
        msk[s + 1:, 0] = 1.0
        in_maps.append({
            "x_d": np.ascontiguousarray(X[s * R:(s + 1) * R]),
            "cnt_d": np.ascontiguousarray(
                cnt_rows[s * R:(s + 1) * R].reshape(R, 1)),
            "mask_d": msk,
        })
    return in_maps


def kernel(log_h, durations, events):
    global LAST_EXEC_TIME_NS, LAST_RESULTS
    from concourse.bass_utils import run_bass_kernel_spmd

    assert log_h.shape == (N,) and durations.shape == (N,)

    d64 = durations.astype(np.int64, copy=False)
    e64 = events.astype(np.int64, copy=False)
    cntE = np.bincount(d64[e64 == 1], minlength=K)
    cntO = np.bincount(d64[e64 == 0], minlength=K)
    C1 = max(C1_DEFAULT, int(-(-cntE.max() // 16) * 16))
    C0 = max(C0_DEFAULT, int(-(-cntO.max() // 16) * 16))

    if (C1, C0) not in _CACHE:
        _CACHE[(C1, C0)] = _build_program(C1, C0)
    nc = _CACHE[(C1, C0)]

    in_maps = _shard_inputs(log_h, durations, events, C1, C0)
    res = run_bass_kernel_spmd(
        nc, in_maps, core_ids=list(range(NCORES)), trace=TRACE,
        trace_cores=None)
    LAST_EXEC_TIME_NS = res.exec_time_ns
    LAST_RESULTS = res
    mse = res.results[0]["mse_d"][0, 0]
    return np.asarray(mse, dtype=np.float32).reshape(())

